# revision 34
# baseline (speedup 1.0000x reference)
"""Distributed 2-layer GCN (DGL GraphConv x2 + ReLU) on 8 Trainium2 NeuronCores.

Strategy (1D dst-node partitioning):
  - Core k owns dst nodes [k*12500, (k+1)*12500). Host buckets edges by dst
    partition and sorts by dst block (128 dst nodes per block).
  - Layer 1: the message rows hpre[src] (hpre = h * out_norm, bf16) depend
    only on the inputs, so the host pre-gathers them into a per-core slot
    stream laid out [128 part, chunk, feat] per dst block. The device
    streams each block's slab contiguously (HWDGE, line rate) and
    segment-sums via one-hot matmuls into PSUM (aggT[f,d]), then
    * in_norm, @W1, +b1, relu, @W2, * out_norm -> p2 shard (bf16, padded
    to 128 cols so layer-2 dma_gather rows are 256B).
  - The p2 AllGather is split into 4 sub-collectives over a permuted
    p2_full row layout (see below) so it overlaps the L1 tail and L2 head.
  - Layer 2: p2 depends on runtime values, so it uses dma_gather
    (SWDGE, ~2.3 ns/row): edges sorted by (dst block, src bucket), gather
    p2_full[src] rows + one-hot matmul segment-sum, * in_norm, + b2 ->
    output shard.
  - Quotas (chunks per block / per (group, bucket)) are max-reduced over
    cores so the SPMD instruction stream is identical on all cores; slack
    slots carry dstloc=999 so their one-hot column is all-zero.
  - All small per-core constant tensors are packed into two blobs (one
    int16/bf16, one f32) because each extra input handle costs ~0.1 ms of
    per-call dispatch overhead on this runtime.
"""

import numpy as np
import ml_dtypes

N, E, IN, HID, OUT = 100000, 1600000, 128, 256, 64
NCORES = 8
NLOC = N // NCORES            # 12500
P = 128
NBLK = (NLOC + P - 1) // P    # 98
LAST_ROWS = NLOC - (NBLK - 1) * P  # 84
BF16 = ml_dtypes.bfloat16
NBUCK = 4
BUCK = 25000                  # L2 bucket size (int16-safe gather indices)
SUB = NLOC // NBUCK           # 3125: p2 sub-shard rows per sub-AllGather
GB = 4                        # dst-blocks per L2 gather group
SUBMAX = 14                   # max chunks per dma_gather (SWDGE ring: <=121
                              # descs per SDMA engine)
# p2_full row layout is PERMUTED so that each of the 4 sub-AllGathers
# produces one contiguous 25000-row bucket: node n (owner k=n//NLOC,
# local l=n%NLOC) lands at row (l//SUB)*8*SUB + k*SUB + (l%SUB). Bucket
# t = rows [t*25000, (t+1)*25000) = sub-AllGather t's output, so layer-2
# gathers for bucket t only wait on sub-collective t (which itself only
# waits on the layer-1 blocks producing shard rows [t*SUB, (t+1)*SUB)).


def _blob_layout(C1, C2, iota_w):
    """Column offsets of the packed constant blobs."""
    o16 = {}
    pos = 0
    TOT2 = C2 * P
    for name, width in (("idx", TOT2 // 16), ("dstloc1", C1),
                        ("dstloc2", C2), ("iotaw", iota_w * P),
                        ("w1", HID), ("w2p", 2 * OUT)):
        o16[name] = (pos, width)
        pos += width
    w16 = pos
    o32 = {}
    pos = 0
    for name, width in (("b1p", 2), ("b2bc", OUT), ("ninT", NBLK * P),
                        ("nincol", NBLK), ("noutcol", NBLK)):
        o32[name] = (pos, width)
        pos += width
    return o16, w16, o32, pos


def _host_prep(h, src, dst, W1, b1, W2, b2):
    deg_in = np.bincount(dst, minlength=N)
    deg_out = np.bincount(src, minlength=N)
    nin = (np.clip(deg_in, 1.0, None) ** -0.5).astype(np.float32)
    nout = (np.clip(deg_out, 1.0, None) ** -0.5).astype(np.float32)

    hpre = (h.astype(np.float32) * nout[:, None]).astype(BF16)

    ngrp = -(-NBLK // GB)
    grp_nb = [min(GB, NBLK - g * GB) for g in range(ngrp)]

    cnt1 = np.zeros((NCORES, NBLK), np.int64)            # L1: per dst block
    cnt2 = np.zeros((NCORES, NBLK, NBUCK), np.int64)     # L2: (block, bucket)
    edges1, edges2 = [], []
    for k in range(NCORES):
        sel = (dst // NLOC) == k
        es = src[sel].astype(np.int64)
        ed = (dst[sel] - k * NLOC).astype(np.int64)
        o1 = np.argsort(ed // P, kind="stable")
        es1, ed1 = es[o1], ed[o1]
        cnt1[k] = np.bincount(ed1 // P, minlength=NBLK)
        edges1.append((es1, ed1))
        # L2: bucket by the permuted p2_full row (see layout comment above)
        sk = es // NLOC
        sl = es % NLOC
        sbu = sl // SUB                      # bucket = sub-AllGather index
        sidx = sk * SUB + sl % SUB           # row within bucket (int16-safe)
        key = (ed // P) * NBUCK + sbu
        o2 = np.argsort(key, kind="stable")
        cnt2[k] = np.bincount(key[o2], minlength=NBLK * NBUCK).reshape(NBLK, NBUCK)
        edges2.append((sidx[o2], ed[o2]))

    # L1 quota: chunks per dst block, max over cores
    q1 = np.maximum(1, -(-cnt1.max(axis=0) // P))        # [NBLK]
    C1 = int(q1.sum())
    c0_1 = np.concatenate([[0], np.cumsum(q1)])

    # L2 quota per (group, bucket): max over cores and blocks-in-group
    Q2 = np.zeros((ngrp, NBUCK), np.int64)
    for g in range(ngrp):
        b0, b1_ = g * GB, min((g + 1) * GB, NBLK)
        Q2[g] = np.maximum(1, -(-cnt2[:, b0:b1_, :].max(axis=(0, 1)) // P))
    C2 = int(sum(grp_nb[g] * Q2[g].sum() for g in range(ngrp)))
    TOT2 = C2 * P
    maxq1 = int(q1.max())
    max_slab2 = GB * int(Q2.max())
    iota_w = max(maxq1, max_slab2)
    o16, w16, o32, w32 = _blob_layout(C1, C2, iota_w)

    iotaw = np.tile(np.tile(np.arange(P, dtype=np.float32),
                            (P, 1)).astype(BF16), (1, iota_w))

    in_maps = []
    for k in range(NCORES):
        # ---- L1: pre-gathered message stream + dstloc
        es1, ed1 = edges1[k]
        bstart = np.concatenate([[0], np.cumsum(cnt1[k])])
        b_of_e = ed1 // P
        pos = np.arange(len(ed1)) - bstart[b_of_e]
        slots1 = (c0_1[b_of_e] * P + pos).astype(np.int64)
        m1 = np.zeros((C1 * P, IN), BF16)
        m1[slots1] = hpre[es1]
        # [C1*P, IN] -> [128 part, C1, IN]: partition-major so each
        # partition's slab read is contiguous
        m1 = np.ascontiguousarray(
            m1.reshape(C1, P, IN).transpose(1, 0, 2).reshape(P, C1 * IN))
        dst1 = np.full(C1 * P, 999.0, np.float32)
        dst1[slots1] = ed1 % P
        dst1 = np.ascontiguousarray(dst1.reshape(C1, P).T).astype(BF16)

        # ---- L2: gather indices + dstloc
        es2, ed2 = edges2[k]
        idx = np.zeros(TOT2, np.int32)
        dst2 = np.full(TOT2, 999.0, np.float32)
        starts = np.concatenate([[0], np.cumsum(cnt2[k].reshape(-1))]).astype(np.int64)
        posn = 0
        for g in range(ngrp):
            for t in range(NBUCK):
                for bl in range(grp_nb[g]):
                    b = g * GB + bl
                    n_bt = int(cnt2[k, b, t])
                    s = int(starts[b * NBUCK + t])
                    idx[posn : posn + n_bt] = es2[s : s + n_bt]
                    dst2[posn : posn + n_bt] = ed2[s : s + n_bt] % P
                    posn += int(Q2[g, t]) * P
        assert posn == TOT2
        # wrap idx int16: slot j of each gather at [j%16, j//16]
        wrap = np.tile(idx.astype(np.int16).reshape(-1, 16).T, (8, 1))
        dst2 = np.ascontiguousarray(dst2.reshape(-1, P).T).astype(BF16)

        nin_loc = nin[k * NLOC : (k + 1) * NLOC]
        nout_loc = nout[k * NLOC : (k + 1) * NLOC]
        pad = NBLK * P - NLOC
        nin_cols = np.ascontiguousarray(
            np.pad(nin_loc, (0, pad)).reshape(NBLK, P).T, dtype=np.float32)
        nout_cols = np.ascontiguousarray(
            np.pad(nout_loc, (0, pad)).reshape(NBLK, P).T, dtype=np.float32)
        nin_tiled = np.tile(np.pad(nin_loc, (0, pad)), (P, 1)).astype(np.float32)

        blob16 = np.empty((P, w16), np.int16)
        w2p = W2.reshape(2, P, OUT).transpose(1, 0, 2).reshape(P, 2 * OUT)
        for name, arr in (("idx", wrap), ("dstloc1", dst1.view(np.int16)),
                          ("dstloc2", dst2.view(np.int16)),
                          ("iotaw", iotaw.view(np.int16)),
                          ("w1", W1.astype(BF16).view(np.int16)),
                          ("w2p", np.ascontiguousarray(w2p)
                           .astype(BF16).view(np.int16))):
            off, width = o16[name]
            blob16[:, off : off + width] = arr
        blob32 = np.empty((P, w32), np.float32)
        for name, arr in (
                ("b1p", np.ascontiguousarray(b1.reshape(2, P).T,
                                             dtype=np.float32)),
                ("b2bc", np.tile(b2.astype(np.float32), (P, 1))),
                ("ninT", nin_tiled), ("nincol", nin_cols),
                ("noutcol", nout_cols)):
            off, width = o32[name]
            blob32[:, off : off + width] = arr

        in_maps.append({"m1": m1, "blob16": blob16, "blob32": blob32})
    return (q1, Q2), (C1, C2), in_maps


def _build_program(quotas, stages=("l1", "ag", "l2"), repeat=1,
                   single_packet=False, x2bufs=6):
    import concourse.bacc as bacc
    import concourse.mybir as mybir
    import concourse.tile as tile

    q1, Q2 = quotas
    f32 = mybir.dt.float32
    bf16 = mybir.dt.bfloat16
    i16 = mybir.dt.int16

    ngrp = Q2.shape[0]
    grp_nb = [min(GB, NBLK - g * GB) for g in range(ngrp)]
    C1 = int(q1.sum())
    c0_1 = np.concatenate([[0], np.cumsum(q1)])
    C2 = int(sum(grp_nb[g] * Q2[g].sum() for g in range(ngrp)))
    TOT2 = C2 * P
    maxq1 = int(q1.max())
    max_slab2 = GB * int(Q2.max())
    iota_w = max(maxq1, max_slab2)
    o16, w16, o32, w32 = _blob_layout(C1, C2, iota_w)

    nc = bacc.Bacc(None, num_swdge_queues=4)
    qctr = [0]

    def subsplit(nch):
        nsub = -(-nch // SUBMAX)
        base = nch // nsub
        rem = nch - base * nsub
        return [base + (1 if i < rem else 0) for i in range(nsub)]

    sizes = set()
    for g in range(ngrp):
        for t in range(NBUCK):
            for s in subsplit(grp_nb[g] * int(Q2[g, t])):
                sizes.add(s * P)
    size_regs = {s: nc.gpsimd.to_reg(s) for s in sorted(sizes)}

    m1_d = nc.dram_tensor("m1", [P, C1 * IN], bf16, kind="ExternalInput")
    b16_d = nc.dram_tensor("blob16", [P, w16], i16, kind="ExternalInput")
    b32_d = nc.dram_tensor("blob32", [P, w32], f32, kind="ExternalInput")
    out_d = nc.dram_tensor("out", [NLOC, OUT], f32, kind="ExternalOutput")

    with tile.TileContext(nc) as tc:
        with (
            tc.tile_pool(name="const", bufs=1) as constp,
            tc.tile_pool(name="dram", bufs=1, space="DRAM") as dramp,
            tc.tile_pool(name="x1", bufs=3) as x1p,
            tc.tile_pool(name="x2", bufs=x2bufs) as x2p,
            tc.tile_pool(name="mblk", bufs=3) as mp,
            tc.tile_pool(name="work", bufs=3) as wp,
            tc.tile_pool(name="acc", bufs=1) as accp,
            tc.tile_pool(name="pa", bufs=GB, space="PSUM") as pap,
            tc.tile_pool(name="pz", bufs=2, space="PSUM") as pzp,
            tc.tile_pool(name="pp", bufs=2, space="PSUM") as ppp,
        ):
            b16_sb = constp.tile([P, w16], i16, tag="b16")
            nc.sync.dma_start(out=b16_sb[:], in_=b16_d[:])
            b32_sb = constp.tile([P, w32], f32, tag="b32")
            nc.sync.dma_start(out=b32_sb[:], in_=b32_d[:])

            def s16(name, a, b_, cast=True):
                off, width = o16[name]
                assert 0 <= a and b_ <= width
                ap = b16_sb[:, off + a : off + b_]
                return ap.bitcast(bf16) if cast else ap

            def s32(name, a, b_):
                off, width = o32[name]
                assert 0 <= a and b_ <= width
                return b32_sb[:, off + a : off + b_]

            # p2 stored bf16 padded to 128 cols: 256B rows (dma_gather
            # needs elem_size % 256B == 0); pad half is never read.
            p2_shard = dramp.tile([NLOC, 2 * OUT], bf16, tag="p2s")
            p2_full = dramp.tile([N, 2 * OUT], bf16, tag="p2f")

            def body(_rep):
                # ---------- layer 1: streamed pre-gathered messages -------
                for b in range(NBLK if "l1" in stages else 0):
                    qb = int(q1[b])
                    ch0 = int(c0_1[b])
                    rows = P if b < NBLK - 1 else LAST_ROWS
                    xs = x1p.tile([P, maxq1 * IN], bf16, tag="xs")
                    nc.sync.dma_start(
                        out=xs[:, : qb * IN],
                        in_=m1_d[:, ch0 * IN : (ch0 + qb) * IN])
                    mb = mp.tile([P, iota_w * P], bf16, tag="m")
                    nc.vector.tensor_tensor(
                        out=mb[:, : qb * P].rearrange("p (c d) -> p c d", d=P),
                        in0=s16("iotaw", 0, qb * P)
                            .rearrange("p (c d) -> p c d", d=P),
                        in1=s16("dstloc1", ch0, ch0 + qb)
                            .rearrange("p (c one) -> p c one", one=1)
                            .to_broadcast([P, qb, P]),
                        op=mybir.AluOpType.is_equal,
                    )
                    agg_psum = pap.tile([P, P], f32, tag="pa",
                                        name=f"pa_l1_{b}_r{_rep}")
                    for c in range(qb):
                        nc.tensor.matmul(
                            agg_psum[:],
                            lhsT=xs[:, c * IN : (c + 1) * IN],
                            rhs=mb[:, c * P : (c + 1) * P],
                            start=(c == 0), stop=(c == qb - 1),
                        )
                    # epilogue: * nin, @W1 + b1, relu, @W2, * nout
                    # (weights/activations bf16: ~4x faster on PE than fp32,
                    # well within the 2e-2 tolerance)
                    aggs = wp.tile([P, P], bf16, tag="aggs")
                    nc.vector.tensor_tensor(
                        out=aggs[:], in0=agg_psum[:],
                        in1=s32("ninT", b * P, (b + 1) * P),
                        op=mybir.AluOpType.mult,
                    )
                    x1a = wp.tile([P, P], bf16, tag="x1a")
                    x1b = wp.tile([P, P], bf16, tag="x1b")
                    for hh, xt in ((0, x1a), (1, x1b)):
                        pz = pzp.tile([P, P], f32, tag="pz")
                        nc.tensor.matmul(
                            pz[:], lhsT=s16("w1", hh * P, (hh + 1) * P),
                            rhs=aggs[:], start=True, stop=True,
                        )
                        nc.scalar.activation(
                            out=xt[:], in_=pz[:],
                            func=mybir.ActivationFunctionType.Relu,
                            bias=s32("b1p", hh, hh + 1), scale=1.0,
                        )
                    pp = ppp.tile([P, OUT], f32, tag="pp")
                    nc.tensor.matmul(pp[:], lhsT=x1a[:], rhs=s16("w2p", 0, OUT),
                                     start=True, stop=False)
                    nc.tensor.matmul(pp[:], lhsT=x1b[:],
                                     rhs=s16("w2p", OUT, 2 * OUT),
                                     start=False, stop=True)
                    p2s = wp.tile([P, 2 * OUT], bf16, tag="p2s")
                    nc.vector.tensor_scalar(
                        out=p2s[:, :OUT], in0=pp[:],
                        scalar1=s32("noutcol", b, b + 1), scalar2=None,
                        op0=mybir.AluOpType.mult,
                    )
                    nc.vector.memset(p2s[:, OUT:], 0.0)
                    nc.sync.dma_start(
                        out=p2_shard[b * P : b * P + rows, :],
                        in_=p2s[:rows, :])

                # ---------- layer 2: dma_gather from p2_full --------------
                # Bucket-major (t outer): pass t's gathers only depend on
                # sub-AllGather t, which only depends on the first (t+1)/4
                # of layer 1 — so L2 gathers overlap the back of L1. Block
                # partials accumulate in SBUF (agg2) across passes.
                elem = 2 * OUT
                l2_mode = ("l2" if "l2" in stages else
                           "l2m" if "l2m" in stages else
                           "l2g" if "l2g" in stages else None)
                compute = l2_mode == "l2"
                do_masks = l2_mode in ("l2", "l2m")
                agg2 = None
                if compute:
                    agg2 = accp.tile([P, NBLK * OUT], f32, tag="agg2",
                                     name=f"agg2_r{_rep}")
                # chunk/slot offset of (g, t) in the host slot layout
                gch0 = np.concatenate(
                    [[0], np.cumsum([grp_nb[g] * int(Q2[g].sum())
                                     for g in range(ngrp)])])
                for t in range(NBUCK if l2_mode or "ag" in stages else 0):
                    if "ag" in stages:
                        nc.gpsimd.collective_compute(
                            "AllGather",
                            mybir.AluOpType.bypass,
                            replica_groups=[list(range(NCORES))],
                            ins=[p2_shard[t * SUB : (t + 1) * SUB, :].opt()],
                            outs=[p2_full[t * NCORES * SUB :
                                          (t + 1) * NCORES * SUB, :].opt()],
                        )
                    tab_ap = p2_full[t * BUCK : (t + 1) * BUCK, :]
                    for g in range(ngrp if l2_mode else 0):
                        nb = grp_nb[g]
                        b0 = g * GB
                        ch_base = int(gch0[g]) + nb * int(Q2[g, :t].sum())
                        sl_base = ch_base * P
                        nch = nb * int(Q2[g, t])
                        xg = x2p.tile([P, max_slab2 * elem], bf16, tag="xg")
                        sub0 = 0
                        for snch in subsplit(nch):
                            sni = snch * P
                            sb0 = sl_base + sub0 * P
                            nc.gpsimd.dma_gather(
                                out_ap=xg[:, sub0 * elem : (sub0 + snch) * elem]
                                    .rearrange("p (c d) -> p c d", d=elem),
                                in_ap=tab_ap,
                                idxs_ap=s16("idx", sb0 // 16, (sb0 + sni) // 16,
                                            cast=False),
                                num_idxs=sni,
                                num_idxs_reg=size_regs[sni],
                                elem_size=elem,
                                single_packet=single_packet,
                                queue_num=qctr[0] % 4,
                            )
                            qctr[0] += 1
                            sub0 += snch
                        if do_masks:
                            mb = mp.tile([P, iota_w * P], bf16, tag="m")
                            nc.vector.tensor_tensor(
                                out=mb[:, : nch * P]
                                    .rearrange("p (c d) -> p c d", d=P),
                                in0=s16("iotaw", 0, nch * P)
                                    .rearrange("p (c d) -> p c d", d=P),
                                in1=s16("dstloc2", ch_base, ch_base + nch)
                                    .rearrange("p (c one) -> p c one", one=1)
                                    .to_broadcast([P, nch, P]),
                                op=mybir.AluOpType.is_equal,
                            )
                        if not compute:
                            continue
                        for bl in range(nb):
                            b = b0 + bl
                            pt = pap.tile([P, OUT], f32, tag="pa",
                                          name=f"pa_l2_t{t}_g{g}_{bl}_r{_rep}")
                            q = int(Q2[g, t])
                            for c in range(q):
                                ci = bl * q + c
                                nc.tensor.matmul(
                                    pt[:],
                                    lhsT=mb[:, ci * P : (ci + 1) * P],
                                    rhs=xg[:, ci * elem : ci * elem + OUT],
                                    start=(c == 0), stop=(c == q - 1),
                                )
                            acc = agg2[:, b * OUT : (b + 1) * OUT]
                            if t == 0:
                                nc.scalar.copy(out=acc, in_=pt[:])
                            else:
                                nc.vector.tensor_tensor(
                                    out=acc, in0=acc, in1=pt[:],
                                    op=mybir.AluOpType.add,
                                )
                for b in range(NBLK if compute else 0):
                    rows = P if b < NBLK - 1 else LAST_ROWS
                    outs = wp.tile([P, OUT], f32, tag="outs")
                    nc.vector.tensor_scalar(
                        out=outs[:], in0=agg2[:, b * OUT : (b + 1) * OUT],
                        scalar1=s32("nincol", b, b + 1), scalar2=None,
                        op0=mybir.AluOpType.mult,
                    )
                    nc.vector.tensor_tensor(
                        out=outs[:], in0=outs[:], in1=s32("b2bc", 0, OUT),
                        op=mybir.AluOpType.add,
                    )
                    nc.sync.dma_start(
                        out=out_d[b * P : b * P + rows, :],
                        in_=outs[:rows, :])

            for _rep in range(repeat):
                body(_rep)

    nc.finalize()
    return nc


def run_on_device(in_maps, quotas, trace=False):
    from concourse.bass_utils import run_bass_kernel_spmd

    nc = _build_program(quotas)
    return run_bass_kernel_spmd(nc, in_maps, core_ids=list(range(NCORES)),
                                trace=trace)


def kernel(h, src, dst, W1, b1, W2, b2):
    h = np.asarray(h, dtype=np.float32)
    src = np.asarray(src, dtype=np.int32)
    dst = np.asarray(dst, dtype=np.int32)
    W1 = np.asarray(W1, dtype=np.float32)
    b1 = np.asarray(b1, dtype=np.float32)
    W2 = np.asarray(W2, dtype=np.float32)
    b2 = np.asarray(b2, dtype=np.float32)

    quotas, sizes, in_maps = _host_prep(h, src, dst, W1, b1, W2, b2)
    res = run_on_device(in_maps, quotas)
    shards = [r["out"].astype(np.float32) for r in res.results]
    return np.concatenate(shards, axis=0)


# revision 38
# speedup vs baseline: 1.4008x; 1.4008x over previous
"""Distributed 2-layer GCN (DGL GraphConv x2 + ReLU) on 8 Trainium2 NeuronCores.

Strategy (1D dst-node partitioning):
  - Core k owns dst nodes [k*12500, (k+1)*12500). Host buckets edges by dst
    partition and sorts by dst block (128 dst nodes per block).
  - Layer 1: the message rows hpre[src] (hpre = h * out_norm, bf16) depend
    only on the inputs, so the host pre-gathers them into a per-core slot
    stream laid out [128 part, chunk, feat] per dst block. The device
    streams each block's slab contiguously (HWDGE, line rate) and
    segment-sums via one-hot matmuls into PSUM (aggT[f,d]), then
    * in_norm, @W1, +b1, relu, @W2, * out_norm -> p2 shard (bf16, padded
    to 128 cols so layer-2 dma_gather rows are 256B).
  - The p2 AllGather is split into 4 sub-collectives over a permuted
    p2_full row layout (see below) so it overlaps the L1 tail and L2 head.
  - Layer 2: p2 depends on runtime values, so it uses dma_gather
    (SWDGE, ~2.3 ns/row): edges sorted by (dst block, src bucket), gather
    p2_full[src] rows + one-hot matmul segment-sum, * in_norm, + b2 ->
    output shard.
  - Quotas (chunks per block / per (group, bucket)) are max-reduced over
    cores so the SPMD instruction stream is identical on all cores; slack
    slots carry dstloc=999 so their one-hot column is all-zero.
  - All small per-core constant tensors are packed into two blobs (one
    int16/bf16, one f32) because each extra input handle costs ~0.1 ms of
    per-call dispatch overhead on this runtime.
"""

import numpy as np
import ml_dtypes

N, E, IN, HID, OUT = 100000, 1600000, 128, 256, 64
NCORES = 8
NLOC = N // NCORES            # 12500
P = 128
NBLK = (NLOC + P - 1) // P    # 98
LAST_ROWS = NLOC - (NBLK - 1) * P  # 84
BF16 = ml_dtypes.bfloat16
NBUCK = 4
BUCK = 25000                  # L2 bucket size (int16-safe gather indices)
SUB = NLOC // NBUCK           # 3125: p2 sub-shard rows per sub-AllGather
GB = 4                        # dst-blocks per L2 gather group
SUBMAX = 14                   # max chunks per dma_gather (SWDGE ring: <=121
                              # descs per SDMA engine)
# p2_full row layout is PERMUTED so that each of the 4 sub-AllGathers
# produces one contiguous 25000-row bucket: node n (owner k=n//NLOC,
# local l=n%NLOC) lands at row (l//SUB)*8*SUB + k*SUB + (l%SUB). Bucket
# t = rows [t*25000, (t+1)*25000) = sub-AllGather t's output, so layer-2
# gathers for bucket t only wait on sub-collective t (which itself only
# waits on the layer-1 blocks producing shard rows [t*SUB, (t+1)*SUB)).


def _blob_layout(C1, C2, iota_w):
    """Column offsets of the packed constant blobs."""
    o16 = {}
    pos = 0
    TOT2 = C2 * P
    for name, width in (("idx", TOT2 // 16), ("dstloc1", C1),
                        ("dstloc2", C2), ("iotaw", iota_w * P),
                        ("w1", HID), ("w2p", 2 * OUT)):
        o16[name] = (pos, width)
        pos += width
    w16 = pos
    o32 = {}
    pos = 0
    for name, width in (("b1p", 2), ("b2bc", OUT), ("ninT", NBLK * P),
                        ("nincol", NBLK), ("noutcol", NBLK)):
        o32[name] = (pos, width)
        pos += width
    return o16, w16, o32, pos


def _host_prep(h, src, dst, W1, b1, W2, b2):
    deg_in = np.bincount(dst, minlength=N)
    deg_out = np.bincount(src, minlength=N)
    nin = (np.clip(deg_in, 1.0, None) ** -0.5).astype(np.float32)
    nout = (np.clip(deg_out, 1.0, None) ** -0.5).astype(np.float32)

    hpre = (h.astype(np.float32) * nout[:, None]).astype(BF16)

    ngrp = -(-NBLK // GB)
    grp_nb = [min(GB, NBLK - g * GB) for g in range(ngrp)]

    cnt1 = np.zeros((NCORES, NBLK), np.int64)            # L1: per dst block
    cnt2 = np.zeros((NCORES, NBLK, NBUCK), np.int64)     # L2: (block, bucket)
    edges1, edges2 = [], []
    for k in range(NCORES):
        sel = (dst // NLOC) == k
        es = src[sel].astype(np.int64)
        ed = (dst[sel] - k * NLOC).astype(np.int64)
        o1 = np.argsort(ed // P, kind="stable")
        es1, ed1 = es[o1], ed[o1]
        cnt1[k] = np.bincount(ed1 // P, minlength=NBLK)
        edges1.append((es1, ed1))
        # L2: bucket by the permuted p2_full row (see layout comment above)
        sk = es // NLOC
        sl = es % NLOC
        sbu = sl // SUB                      # bucket = sub-AllGather index
        sidx = sk * SUB + sl % SUB           # row within bucket (int16-safe)
        key = (ed // P) * NBUCK + sbu
        o2 = np.argsort(key, kind="stable")
        cnt2[k] = np.bincount(key[o2], minlength=NBLK * NBUCK).reshape(NBLK, NBUCK)
        edges2.append((sidx[o2], ed[o2]))

    # L1 quota: chunks per dst block, max over cores
    q1 = np.maximum(1, -(-cnt1.max(axis=0) // P))        # [NBLK]
    C1 = int(q1.sum())
    c0_1 = np.concatenate([[0], np.cumsum(q1)])

    # L2 quota per (group, bucket): max over cores and blocks-in-group
    Q2 = np.zeros((ngrp, NBUCK), np.int64)
    for g in range(ngrp):
        b0, b1_ = g * GB, min((g + 1) * GB, NBLK)
        Q2[g] = np.maximum(1, -(-cnt2[:, b0:b1_, :].max(axis=(0, 1)) // P))
    C2 = int(sum(grp_nb[g] * Q2[g].sum() for g in range(ngrp)))
    TOT2 = C2 * P
    maxq1 = int(q1.max())
    max_slab2 = GB * int(Q2.max())
    iota_w = max(maxq1, max_slab2)
    o16, w16, o32, w32 = _blob_layout(C1, C2, iota_w)

    iotaw = np.tile(np.tile(np.arange(P, dtype=np.float32),
                            (P, 1)).astype(BF16), (1, iota_w))

    in_maps = []
    for k in range(NCORES):
        # ---- L1: pre-gathered message stream + dstloc
        es1, ed1 = edges1[k]
        bstart = np.concatenate([[0], np.cumsum(cnt1[k])])
        b_of_e = ed1 // P
        pos = np.arange(len(ed1)) - bstart[b_of_e]
        slots1 = (c0_1[b_of_e] * P + pos).astype(np.int64)
        m1 = np.zeros((C1 * P, IN), BF16)
        m1[slots1] = hpre[es1]
        # [C1*P, IN] -> [128 part, C1, IN]: partition-major so each
        # partition's slab read is contiguous
        m1 = np.ascontiguousarray(
            m1.reshape(C1, P, IN).transpose(1, 0, 2).reshape(P, C1 * IN))
        dst1 = np.full(C1 * P, 999.0, np.float32)
        dst1[slots1] = ed1 % P
        dst1 = np.ascontiguousarray(dst1.reshape(C1, P).T).astype(BF16)

        # ---- L2: gather indices + dstloc
        es2, ed2 = edges2[k]
        idx = np.zeros(TOT2, np.int32)
        dst2 = np.full(TOT2, 999.0, np.float32)
        starts = np.concatenate([[0], np.cumsum(cnt2[k].reshape(-1))]).astype(np.int64)
        posn = 0
        for g in range(ngrp):
            for t in range(NBUCK):
                for bl in range(grp_nb[g]):
                    b = g * GB + bl
                    n_bt = int(cnt2[k, b, t])
                    s = int(starts[b * NBUCK + t])
                    idx[posn : posn + n_bt] = es2[s : s + n_bt]
                    dst2[posn : posn + n_bt] = ed2[s : s + n_bt] % P
                    posn += int(Q2[g, t]) * P
        assert posn == TOT2
        # wrap idx int16: slot j of each gather at [j%16, j//16]
        wrap = np.tile(idx.astype(np.int16).reshape(-1, 16).T, (8, 1))
        dst2 = np.ascontiguousarray(dst2.reshape(-1, P).T).astype(BF16)

        nin_loc = nin[k * NLOC : (k + 1) * NLOC]
        nout_loc = nout[k * NLOC : (k + 1) * NLOC]
        pad = NBLK * P - NLOC
        nin_cols = np.ascontiguousarray(
            np.pad(nin_loc, (0, pad)).reshape(NBLK, P).T, dtype=np.float32)
        nout_cols = np.ascontiguousarray(
            np.pad(nout_loc, (0, pad)).reshape(NBLK, P).T, dtype=np.float32)
        nin_tiled = np.tile(np.pad(nin_loc, (0, pad)), (P, 1)).astype(np.float32)

        blob16 = np.empty((P, w16), np.int16)
        w2p = W2.reshape(2, P, OUT).transpose(1, 0, 2).reshape(P, 2 * OUT)
        for name, arr in (("idx", wrap), ("dstloc1", dst1.view(np.int16)),
                          ("dstloc2", dst2.view(np.int16)),
                          ("iotaw", iotaw.view(np.int16)),
                          ("w1", W1.astype(BF16).view(np.int16)),
                          ("w2p", np.ascontiguousarray(w2p)
                           .astype(BF16).view(np.int16))):
            off, width = o16[name]
            blob16[:, off : off + width] = arr
        blob32 = np.empty((P, w32), np.float32)
        for name, arr in (
                ("b1p", np.ascontiguousarray(b1.reshape(2, P).T,
                                             dtype=np.float32)),
                ("b2bc", np.tile(b2.astype(np.float32), (P, 1))),
                ("ninT", nin_tiled), ("nincol", nin_cols),
                ("noutcol", nout_cols)):
            off, width = o32[name]
            blob32[:, off : off + width] = arr

        in_maps.append({"m1": m1, "blob16": blob16, "blob32": blob32})
    return (q1, Q2), (C1, C2), in_maps


def _build_program(quotas, stages=("l1", "ag", "l2"), repeat=1,
                   single_packet=False, x2bufs=6):
    import concourse.bacc as bacc
    import concourse.mybir as mybir
    import concourse.tile as tile

    q1, Q2 = quotas
    f32 = mybir.dt.float32
    bf16 = mybir.dt.bfloat16
    i16 = mybir.dt.int16

    ngrp = Q2.shape[0]
    grp_nb = [min(GB, NBLK - g * GB) for g in range(ngrp)]
    C1 = int(q1.sum())
    c0_1 = np.concatenate([[0], np.cumsum(q1)])
    C2 = int(sum(grp_nb[g] * Q2[g].sum() for g in range(ngrp)))
    TOT2 = C2 * P
    maxq1 = int(q1.max())
    max_slab2 = GB * int(Q2.max())
    iota_w = max(maxq1, max_slab2)
    o16, w16, o32, w32 = _blob_layout(C1, C2, iota_w)

    nc = bacc.Bacc(None, num_swdge_queues=4)
    qctr = [0]

    def subsplit(nch):
        nsub = -(-nch // SUBMAX)
        base = nch // nsub
        rem = nch - base * nsub
        return [base + (1 if i < rem else 0) for i in range(nsub)]

    sizes = set()
    for g in range(ngrp):
        for t in range(NBUCK):
            for s in subsplit(grp_nb[g] * int(Q2[g, t])):
                sizes.add(s * P)
    size_regs = {s: nc.gpsimd.to_reg(s) for s in sorted(sizes)}

    m1_d = nc.dram_tensor("m1", [P, C1 * IN], bf16, kind="ExternalInput")
    b16_d = nc.dram_tensor("blob16", [P, w16], i16, kind="ExternalInput")
    b32_d = nc.dram_tensor("blob32", [P, w32], f32, kind="ExternalInput")
    out_d = nc.dram_tensor("out", [NLOC, OUT], f32, kind="ExternalOutput")

    with tile.TileContext(nc) as tc:
        with (
            tc.tile_pool(name="const", bufs=1) as constp,
            tc.tile_pool(name="dram", bufs=1, space="DRAM") as dramp,
            tc.tile_pool(name="x1", bufs=3) as x1p,
            tc.tile_pool(name="x2", bufs=x2bufs) as x2p,
            tc.tile_pool(name="mblk", bufs=3) as mp,
            tc.tile_pool(name="work", bufs=3) as wp,
            tc.tile_pool(name="acc", bufs=1) as accp,
            tc.tile_pool(name="pa", bufs=GB, space="PSUM") as pap,
            tc.tile_pool(name="pz", bufs=2, space="PSUM") as pzp,
            tc.tile_pool(name="pp", bufs=2, space="PSUM") as ppp,
        ):
            b16_sb = constp.tile([P, w16], i16, tag="b16")
            nc.sync.dma_start(out=b16_sb[:], in_=b16_d[:])
            b32_sb = constp.tile([P, w32], f32, tag="b32")
            nc.sync.dma_start(out=b32_sb[:], in_=b32_d[:])

            def s16(name, a, b_, cast=True):
                off, width = o16[name]
                assert 0 <= a and b_ <= width
                ap = b16_sb[:, off + a : off + b_]
                return ap.bitcast(bf16) if cast else ap

            def s32(name, a, b_):
                off, width = o32[name]
                assert 0 <= a and b_ <= width
                return b32_sb[:, off + a : off + b_]

            # p2 stored bf16 padded to 128 cols: 256B rows (dma_gather
            # needs elem_size % 256B == 0); pad half is never read.
            p2_shard = dramp.tile([NLOC, 2 * OUT], bf16, tag="p2s")
            p2_full = dramp.tile([N, 2 * OUT], bf16, tag="p2f")

            # block whose p2_shard write completes sub-shard t
            ag_after = {((t + 1) * SUB - 1) // P: t for t in range(NBUCK)}

            def body(_rep):
                # ---------- layer 1: streamed pre-gathered messages -------
                for b in range(NBLK if "l1" in stages else 0):
                    qb = int(q1[b])
                    ch0 = int(c0_1[b])
                    rows = P if b < NBLK - 1 else LAST_ROWS
                    xs = x1p.tile([P, maxq1 * IN], bf16, tag="xs")
                    nc.sync.dma_start(
                        out=xs[:, : qb * IN],
                        in_=m1_d[:, ch0 * IN : (ch0 + qb) * IN])
                    mb = mp.tile([P, iota_w * P], bf16, tag="m")
                    nc.vector.tensor_tensor(
                        out=mb[:, : qb * P].rearrange("p (c d) -> p c d", d=P),
                        in0=s16("iotaw", 0, qb * P)
                            .rearrange("p (c d) -> p c d", d=P),
                        in1=s16("dstloc1", ch0, ch0 + qb)
                            .rearrange("p (c one) -> p c one", one=1)
                            .to_broadcast([P, qb, P]),
                        op=mybir.AluOpType.is_equal,
                    )
                    agg_psum = pap.tile([P, P], f32, tag="pa",
                                        name=f"pa_l1_{b}_r{_rep}")
                    for c in range(qb):
                        nc.tensor.matmul(
                            agg_psum[:],
                            lhsT=xs[:, c * IN : (c + 1) * IN],
                            rhs=mb[:, c * P : (c + 1) * P],
                            start=(c == 0), stop=(c == qb - 1),
                        )
                    # epilogue: * nin, @W1 + b1, relu, @W2, * nout
                    # (weights/activations bf16: ~4x faster on PE than fp32,
                    # well within the 2e-2 tolerance)
                    aggs = wp.tile([P, P], bf16, tag="aggs")
                    nc.vector.tensor_tensor(
                        out=aggs[:], in0=agg_psum[:],
                        in1=s32("ninT", b * P, (b + 1) * P),
                        op=mybir.AluOpType.mult,
                    )
                    x1a = wp.tile([P, P], bf16, tag="x1a")
                    x1b = wp.tile([P, P], bf16, tag="x1b")
                    for hh, xt in ((0, x1a), (1, x1b)):
                        pz = pzp.tile([P, P], f32, tag="pz")
                        nc.tensor.matmul(
                            pz[:], lhsT=s16("w1", hh * P, (hh + 1) * P),
                            rhs=aggs[:], start=True, stop=True,
                        )
                        nc.scalar.activation(
                            out=xt[:], in_=pz[:],
                            func=mybir.ActivationFunctionType.Relu,
                            bias=s32("b1p", hh, hh + 1), scale=1.0,
                        )
                    pp = ppp.tile([P, OUT], f32, tag="pp")
                    nc.tensor.matmul(pp[:], lhsT=x1a[:], rhs=s16("w2p", 0, OUT),
                                     start=True, stop=False)
                    nc.tensor.matmul(pp[:], lhsT=x1b[:],
                                     rhs=s16("w2p", OUT, 2 * OUT),
                                     start=False, stop=True)
                    p2s = wp.tile([P, 2 * OUT], bf16, tag="p2s")
                    nc.vector.tensor_scalar(
                        out=p2s[:, :OUT], in0=pp[:],
                        scalar1=s32("noutcol", b, b + 1), scalar2=None,
                        op0=mybir.AluOpType.mult,
                    )
                    nc.vector.memset(p2s[:, OUT:], 0.0)
                    nc.sync.dma_start(
                        out=p2_shard[b * P : b * P + rows, :],
                        in_=p2s[:rows, :])
                    # fire sub-AllGather t as soon as its quarter of the
                    # shard is written, so collectives run during L1
                    if "ag" in stages and b in ag_after:
                        t = ag_after[b]
                        nc.gpsimd.collective_compute(
                            "AllGather",
                            mybir.AluOpType.bypass,
                            replica_groups=[list(range(NCORES))],
                            ins=[p2_shard[t * SUB : (t + 1) * SUB, :].opt()],
                            outs=[p2_full[t * NCORES * SUB :
                                          (t + 1) * NCORES * SUB, :].opt()],
                        )

                if "ag" in stages and "l1" not in stages:
                    for t in range(NBUCK):
                        nc.gpsimd.collective_compute(
                            "AllGather",
                            mybir.AluOpType.bypass,
                            replica_groups=[list(range(NCORES))],
                            ins=[p2_shard[t * SUB : (t + 1) * SUB, :].opt()],
                            outs=[p2_full[t * NCORES * SUB :
                                          (t + 1) * NCORES * SUB, :].opt()],
                        )

                # ---------- layer 2: dma_gather from p2_full --------------
                elem = 2 * OUT
                slot0 = 0
                ch0 = 0
                l2_mode = ("l2" if "l2" in stages else
                           "l2m" if "l2m" in stages else
                           "l2g" if "l2g" in stages else None)
                for g in range(ngrp if l2_mode else 0):
                    nb = grp_nb[g]
                    b0 = g * GB
                    compute = l2_mode == "l2"
                    masks = l2_mode in ("l2", "l2m")
                    aggs_psum = [
                        pap.tile([P, OUT], f32, tag="pa",
                                 name=f"pa_l2_g{g}_{bl}_r{_rep}")
                        for bl in range(nb)
                    ] if compute else None
                    qsum = int(Q2[g].sum())
                    ch_base = ch0
                    sl_base = slot0
                    for t in range(NBUCK):
                        nch = nb * int(Q2[g, t])
                        ni = nch * P
                        xg = x2p.tile([P, max_slab2 * elem], bf16, tag="xg")
                        tab_ap = p2_full[t * BUCK : (t + 1) * BUCK, :]
                        sub0 = 0
                        for snch in subsplit(nch):
                            sni = snch * P
                            sb0 = sl_base + sub0 * P
                            nc.gpsimd.dma_gather(
                                out_ap=xg[:, sub0 * elem : (sub0 + snch) * elem]
                                    .rearrange("p (c d) -> p c d", d=elem),
                                in_ap=tab_ap,
                                idxs_ap=s16("idx", sb0 // 16, (sb0 + sni) // 16,
                                            cast=False),
                                num_idxs=sni,
                                num_idxs_reg=size_regs[sni],
                                elem_size=elem,
                                single_packet=single_packet,
                                queue_num=qctr[0] % 4,
                            )
                            qctr[0] += 1
                            sub0 += snch
                        if masks:
                            mb = mp.tile([P, iota_w * P], bf16, tag="m")
                            nc.vector.tensor_tensor(
                                out=mb[:, : nch * P]
                                    .rearrange("p (c d) -> p c d", d=P),
                                in0=s16("iotaw", 0, nch * P)
                                    .rearrange("p (c d) -> p c d", d=P),
                                in1=s16("dstloc2", ch_base, ch_base + nch)
                                    .rearrange("p (c one) -> p c one", one=1)
                                    .to_broadcast([P, nch, P]),
                                op=mybir.AluOpType.is_equal,
                            )
                        if compute:
                            for bl in range(nb):
                                for c in range(int(Q2[g, t])):
                                    ci = bl * int(Q2[g, t]) + c
                                    first = t == 0 and c == 0
                                    last = (t == NBUCK - 1
                                            and c == int(Q2[g, t]) - 1)
                                    nc.tensor.matmul(
                                        aggs_psum[bl][:],
                                        lhsT=mb[:, ci * P : (ci + 1) * P],
                                        rhs=xg[:, ci * elem : ci * elem + OUT],
                                        start=first, stop=last,
                                    )
                        ch_base += nch
                        sl_base += ni
                    ch0 += nb * qsum
                    slot0 += nb * qsum * P
                    for bl in range(nb if compute else 0):
                        b = b0 + bl
                        rows = P if b < NBLK - 1 else LAST_ROWS
                        outs = wp.tile([P, OUT], f32, tag="outs")
                        nc.vector.tensor_scalar(
                            out=outs[:], in0=aggs_psum[bl][:],
                            scalar1=s32("nincol", b, b + 1), scalar2=None,
                            op0=mybir.AluOpType.mult,
                        )
                        nc.vector.tensor_tensor(
                            out=outs[:], in0=outs[:], in1=s32("b2bc", 0, OUT),
                            op=mybir.AluOpType.add,
                        )
                        nc.sync.dma_start(
                            out=out_d[b * P : b * P + rows, :],
                            in_=outs[:rows, :])

            for _rep in range(repeat):
                body(_rep)

    nc.finalize()
    return nc


def run_on_device(in_maps, quotas, trace=False):
    from concourse.bass_utils import run_bass_kernel_spmd

    nc = _build_program(quotas)
    return run_bass_kernel_spmd(nc, in_maps, core_ids=list(range(NCORES)),
                                trace=trace)


def kernel(h, src, dst, W1, b1, W2, b2):
    h = np.asarray(h, dtype=np.float32)
    src = np.asarray(src, dtype=np.int32)
    dst = np.asarray(dst, dtype=np.int32)
    W1 = np.asarray(W1, dtype=np.float32)
    b1 = np.asarray(b1, dtype=np.float32)
    W2 = np.asarray(W2, dtype=np.float32)
    b2 = np.asarray(b2, dtype=np.float32)

    quotas, sizes, in_maps = _host_prep(h, src, dst, W1, b1, W2, b2)
    res = run_on_device(in_maps, quotas)
    shards = [r["out"].astype(np.float32) for r in res.results]
    return np.concatenate(shards, axis=0)


# revision 39
# speedup vs baseline: 1.5276x; 1.0905x over previous
"""Distributed 2-layer GCN (DGL GraphConv x2 + ReLU) on 8 Trainium2 NeuronCores.

Strategy (1D dst-node partitioning):
  - Core k owns dst nodes [k*12500, (k+1)*12500). Host buckets edges by dst
    partition and sorts by dst block (128 dst nodes per block).
  - Layer 1: the message rows hpre[src] (hpre = h * out_norm, bf16) depend
    only on the inputs, so the host pre-gathers them into a per-core slot
    stream laid out [128 part, chunk, feat] per dst block. The device
    streams each block's slab contiguously (HWDGE, line rate) and
    segment-sums via one-hot matmuls into PSUM (aggT[f,d]), then
    * in_norm, @W1, +b1, relu, @W2, * out_norm -> p2 shard (bf16, padded
    to 128 cols so layer-2 dma_gather rows are 256B).
  - The p2 AllGather is split into 4 sub-collectives over a permuted
    p2_full row layout (see below) so it overlaps the L1 tail and L2 head.
  - Layer 2: p2 depends on runtime values, so it uses dma_gather
    (SWDGE, ~2.3 ns/row): edges sorted by (dst block, src bucket), gather
    p2_full[src] rows + one-hot matmul segment-sum, * in_norm, + b2 ->
    output shard.
  - Quotas (chunks per block / per (group, bucket)) are max-reduced over
    cores so the SPMD instruction stream is identical on all cores; slack
    slots carry dstloc=999 so their one-hot column is all-zero.
  - All small per-core constant tensors are packed into two blobs (one
    int16/bf16, one f32) because each extra input handle costs ~0.1 ms of
    per-call dispatch overhead on this runtime.
"""

import numpy as np
import ml_dtypes

N, E, IN, HID, OUT = 100000, 1600000, 128, 256, 64
NCORES = 8
NLOC = N // NCORES            # 12500
P = 128
NBLK = (NLOC + P - 1) // P    # 98
LAST_ROWS = NLOC - (NBLK - 1) * P  # 84
BF16 = ml_dtypes.bfloat16
NBUCK = 4
BUCK = 25000                  # L2 bucket size (int16-safe gather indices)
SUB = NLOC // NBUCK           # 3125: p2 sub-shard rows per sub-AllGather
GB = 4                        # dst-blocks per L2 gather group
SUBMAX = 14                   # max chunks per dma_gather (SWDGE ring: <=121
                              # descs per SDMA engine)
# p2_full row layout is PERMUTED so that each of the 4 sub-AllGathers
# produces one contiguous 25000-row bucket: node n (owner k=n//NLOC,
# local l=n%NLOC) lands at row (l//SUB)*8*SUB + k*SUB + (l%SUB). Bucket
# t = rows [t*25000, (t+1)*25000) = sub-AllGather t's output, so layer-2
# gathers for bucket t only wait on sub-collective t (which itself only
# waits on the layer-1 blocks producing shard rows [t*SUB, (t+1)*SUB)).


def _blob_layout(C1, C2, iota_w):
    """Column offsets of the packed constant blobs."""
    o16 = {}
    pos = 0
    TOT2 = C2 * P
    for name, width in (("idx", TOT2 // 16), ("dstloc1", C1),
                        ("dstloc2", C2), ("iotaw", iota_w * P),
                        ("w1", HID), ("w2p", 2 * OUT)):
        o16[name] = (pos, width)
        pos += width
    w16 = pos
    o32 = {}
    pos = 0
    for name, width in (("b1p", 2), ("b2bc", OUT), ("ninT", NBLK * P),
                        ("nincol", NBLK), ("noutcol", NBLK)):
        o32[name] = (pos, width)
        pos += width
    return o16, w16, o32, pos


def _host_prep(h, src, dst, W1, b1, W2, b2):
    deg_in = np.bincount(dst, minlength=N)
    deg_out = np.bincount(src, minlength=N)
    nin = (np.clip(deg_in, 1.0, None) ** -0.5).astype(np.float32)
    nout = (np.clip(deg_out, 1.0, None) ** -0.5).astype(np.float32)

    hpre = (h.astype(np.float32) * nout[:, None]).astype(BF16)

    ngrp = -(-NBLK // GB)
    grp_nb = [min(GB, NBLK - g * GB) for g in range(ngrp)]

    cnt1 = np.zeros((NCORES, NBLK), np.int64)            # L1: per dst block
    cnt2 = np.zeros((NCORES, NBLK, NBUCK), np.int64)     # L2: (block, bucket)
    edges1, edges2 = [], []
    for k in range(NCORES):
        sel = (dst // NLOC) == k
        es = src[sel].astype(np.int64)
        ed = (dst[sel] - k * NLOC).astype(np.int64)
        o1 = np.argsort(ed // P, kind="stable")
        es1, ed1 = es[o1], ed[o1]
        cnt1[k] = np.bincount(ed1 // P, minlength=NBLK)
        edges1.append((es1, ed1))
        # L2: bucket by the permuted p2_full row (see layout comment above)
        sk = es // NLOC
        sl = es % NLOC
        sbu = sl // SUB                      # bucket = sub-AllGather index
        sidx = sk * SUB + sl % SUB           # row within bucket (int16-safe)
        key = (ed // P) * NBUCK + sbu
        o2 = np.argsort(key, kind="stable")
        cnt2[k] = np.bincount(key[o2], minlength=NBLK * NBUCK).reshape(NBLK, NBUCK)
        edges2.append((sidx[o2], ed[o2]))

    # L1 quota: chunks per dst block, max over cores
    q1 = np.maximum(1, -(-cnt1.max(axis=0) // P))        # [NBLK]
    C1 = int(q1.sum())
    c0_1 = np.concatenate([[0], np.cumsum(q1)])

    # L2 quota per (group, bucket): max over cores and blocks-in-group
    Q2 = np.zeros((ngrp, NBUCK), np.int64)
    for g in range(ngrp):
        b0, b1_ = g * GB, min((g + 1) * GB, NBLK)
        Q2[g] = np.maximum(1, -(-cnt2[:, b0:b1_, :].max(axis=(0, 1)) // P))
    C2 = int(sum(grp_nb[g] * Q2[g].sum() for g in range(ngrp)))
    TOT2 = C2 * P
    maxq1 = int(q1.max())
    max_slab2 = GB * int(Q2.max())
    iota_w = max(maxq1, max_slab2)
    o16, w16, o32, w32 = _blob_layout(C1, C2, iota_w)

    iotaw = np.tile(np.tile(np.arange(P, dtype=np.float32),
                            (P, 1)).astype(BF16), (1, iota_w))

    in_maps = []
    for k in range(NCORES):
        # ---- L1: pre-gathered message stream + dstloc
        es1, ed1 = edges1[k]
        bstart = np.concatenate([[0], np.cumsum(cnt1[k])])
        b_of_e = ed1 // P
        pos = np.arange(len(ed1)) - bstart[b_of_e]
        slots1 = (c0_1[b_of_e] * P + pos).astype(np.int64)
        m1 = np.zeros((C1 * P, IN), BF16)
        m1[slots1] = hpre[es1]
        # [C1*P, IN] -> [128 part, C1, IN]: partition-major so each
        # partition's slab read is contiguous
        m1 = np.ascontiguousarray(
            m1.reshape(C1, P, IN).transpose(1, 0, 2).reshape(P, C1 * IN))
        dst1 = np.full(C1 * P, 999.0, np.float32)
        dst1[slots1] = ed1 % P
        dst1 = np.ascontiguousarray(dst1.reshape(C1, P).T).astype(BF16)

        # ---- L2: gather indices + dstloc
        es2, ed2 = edges2[k]
        idx = np.zeros(TOT2, np.int32)
        dst2 = np.full(TOT2, 999.0, np.float32)
        starts = np.concatenate([[0], np.cumsum(cnt2[k].reshape(-1))]).astype(np.int64)
        posn = 0
        for g in range(ngrp):
            for t in range(NBUCK):
                for bl in range(grp_nb[g]):
                    b = g * GB + bl
                    n_bt = int(cnt2[k, b, t])
                    s = int(starts[b * NBUCK + t])
                    idx[posn : posn + n_bt] = es2[s : s + n_bt]
                    dst2[posn : posn + n_bt] = ed2[s : s + n_bt] % P
                    posn += int(Q2[g, t]) * P
        assert posn == TOT2
        # wrap idx int16: slot j of each gather at [j%16, j//16]
        wrap = np.tile(idx.astype(np.int16).reshape(-1, 16).T, (8, 1))
        dst2 = np.ascontiguousarray(dst2.reshape(-1, P).T).astype(BF16)

        nin_loc = nin[k * NLOC : (k + 1) * NLOC]
        nout_loc = nout[k * NLOC : (k + 1) * NLOC]
        pad = NBLK * P - NLOC
        nin_cols = np.ascontiguousarray(
            np.pad(nin_loc, (0, pad)).reshape(NBLK, P).T, dtype=np.float32)
        nout_cols = np.ascontiguousarray(
            np.pad(nout_loc, (0, pad)).reshape(NBLK, P).T, dtype=np.float32)
        nin_tiled = np.tile(np.pad(nin_loc, (0, pad)), (P, 1)).astype(np.float32)

        blob16 = np.empty((P, w16), np.int16)
        w2p = W2.reshape(2, P, OUT).transpose(1, 0, 2).reshape(P, 2 * OUT)
        for name, arr in (("idx", wrap), ("dstloc1", dst1.view(np.int16)),
                          ("dstloc2", dst2.view(np.int16)),
                          ("iotaw", iotaw.view(np.int16)),
                          ("w1", W1.astype(BF16).view(np.int16)),
                          ("w2p", np.ascontiguousarray(w2p)
                           .astype(BF16).view(np.int16))):
            off, width = o16[name]
            blob16[:, off : off + width] = arr
        blob32 = np.empty((P, w32), np.float32)
        for name, arr in (
                ("b1p", np.ascontiguousarray(b1.reshape(2, P).T,
                                             dtype=np.float32)),
                ("b2bc", np.tile(b2.astype(np.float32), (P, 1))),
                ("ninT", nin_tiled), ("nincol", nin_cols),
                ("noutcol", nout_cols)):
            off, width = o32[name]
            blob32[:, off : off + width] = arr

        in_maps.append({"m1": m1, "blob16": blob16, "blob32": blob32})
    return (q1, Q2), (C1, C2), in_maps


def _build_program(quotas, stages=("l1", "ag", "l2"), repeat=1,
                   single_packet=False, x2bufs=6):
    import concourse.bacc as bacc
    import concourse.mybir as mybir
    import concourse.tile as tile

    q1, Q2 = quotas
    f32 = mybir.dt.float32
    bf16 = mybir.dt.bfloat16
    i16 = mybir.dt.int16

    ngrp = Q2.shape[0]
    grp_nb = [min(GB, NBLK - g * GB) for g in range(ngrp)]
    C1 = int(q1.sum())
    c0_1 = np.concatenate([[0], np.cumsum(q1)])
    C2 = int(sum(grp_nb[g] * Q2[g].sum() for g in range(ngrp)))
    TOT2 = C2 * P
    maxq1 = int(q1.max())
    max_slab2 = GB * int(Q2.max())
    iota_w = max(maxq1, max_slab2)
    o16, w16, o32, w32 = _blob_layout(C1, C2, iota_w)

    nc = bacc.Bacc(None, num_swdge_queues=4)
    qctr = [0]

    def subsplit(nch):
        nsub = -(-nch // SUBMAX)
        base = nch // nsub
        rem = nch - base * nsub
        return [base + (1 if i < rem else 0) for i in range(nsub)]

    sizes = set()
    for g in range(ngrp):
        for t in range(NBUCK):
            for s in subsplit(grp_nb[g] * int(Q2[g, t])):
                sizes.add(s * P)
    size_regs = {s: nc.gpsimd.to_reg(s) for s in sorted(sizes)}

    m1_d = nc.dram_tensor("m1", [P, C1 * IN], bf16, kind="ExternalInput")
    b16_d = nc.dram_tensor("blob16", [P, w16], i16, kind="ExternalInput")
    b32_d = nc.dram_tensor("blob32", [P, w32], f32, kind="ExternalInput")
    out_d = nc.dram_tensor("out", [NLOC, OUT], f32, kind="ExternalOutput")

    with tile.TileContext(nc) as tc:
        with (
            tc.tile_pool(name="const", bufs=1) as constp,
            tc.tile_pool(name="dram", bufs=1, space="DRAM") as dramp,
            tc.tile_pool(name="x1", bufs=3) as x1p,
            tc.tile_pool(name="x2", bufs=x2bufs) as x2p,
            tc.tile_pool(name="mblk", bufs=3) as mp,
            tc.tile_pool(name="work", bufs=3) as wp,
            tc.tile_pool(name="acc", bufs=1) as accp,
            tc.tile_pool(name="pa", bufs=GB, space="PSUM") as pap,
            tc.tile_pool(name="pz", bufs=2, space="PSUM") as pzp,
            tc.tile_pool(name="pp", bufs=2, space="PSUM") as ppp,
        ):
            b16_sb = constp.tile([P, w16], i16, tag="b16")
            nc.sync.dma_start(out=b16_sb[:], in_=b16_d[:])
            b32_sb = constp.tile([P, w32], f32, tag="b32")
            nc.sync.dma_start(out=b32_sb[:], in_=b32_d[:])

            def s16(name, a, b_, cast=True):
                off, width = o16[name]
                assert 0 <= a and b_ <= width
                ap = b16_sb[:, off + a : off + b_]
                return ap.bitcast(bf16) if cast else ap

            def s32(name, a, b_):
                off, width = o32[name]
                assert 0 <= a and b_ <= width
                return b32_sb[:, off + a : off + b_]

            # p2 stored bf16 padded to 128 cols: 256B rows (dma_gather
            # needs elem_size % 256B == 0); pad half is never read.
            p2_shard = dramp.tile([NLOC, 2 * OUT], bf16, tag="p2s")
            p2_full = dramp.tile([N, 2 * OUT], bf16, tag="p2f")

            # block whose p2_shard write completes sub-shard t
            ag_after = {((t + 1) * SUB - 1) // P: t for t in range(NBUCK)}

            def body(_rep):
                # ---------- layer 1: streamed pre-gathered messages -------
                for b in range(NBLK if "l1" in stages else 0):
                    qb = int(q1[b])
                    ch0 = int(c0_1[b])
                    rows = P if b < NBLK - 1 else LAST_ROWS
                    xs = x1p.tile([P, maxq1 * IN], bf16, tag="xs")
                    nc.sync.dma_start(
                        out=xs[:, : qb * IN],
                        in_=m1_d[:, ch0 * IN : (ch0 + qb) * IN])
                    mb = mp.tile([P, iota_w * P], bf16, tag="m")
                    nc.vector.tensor_tensor(
                        out=mb[:, : qb * P].rearrange("p (c d) -> p c d", d=P),
                        in0=s16("iotaw", 0, qb * P)
                            .rearrange("p (c d) -> p c d", d=P),
                        in1=s16("dstloc1", ch0, ch0 + qb)
                            .rearrange("p (c one) -> p c one", one=1)
                            .to_broadcast([P, qb, P]),
                        op=mybir.AluOpType.is_equal,
                    )
                    agg_psum = pap.tile([P, P], f32, tag="pa",
                                        name=f"pa_l1_{b}_r{_rep}")
                    for c in range(qb):
                        nc.tensor.matmul(
                            agg_psum[:],
                            lhsT=xs[:, c * IN : (c + 1) * IN],
                            rhs=mb[:, c * P : (c + 1) * P],
                            start=(c == 0), stop=(c == qb - 1),
                        )
                    # epilogue: * nin, @W1 + b1, relu, @W2, * nout
                    # (weights/activations bf16: ~4x faster on PE than fp32,
                    # well within the 2e-2 tolerance)
                    aggs = wp.tile([P, P], bf16, tag="aggs")
                    nc.vector.tensor_tensor(
                        out=aggs[:], in0=agg_psum[:],
                        in1=s32("ninT", b * P, (b + 1) * P),
                        op=mybir.AluOpType.mult,
                    )
                    x1a = wp.tile([P, P], bf16, tag="x1a")
                    x1b = wp.tile([P, P], bf16, tag="x1b")
                    for hh, xt in ((0, x1a), (1, x1b)):
                        pz = pzp.tile([P, P], f32, tag="pz")
                        nc.tensor.matmul(
                            pz[:], lhsT=s16("w1", hh * P, (hh + 1) * P),
                            rhs=aggs[:], start=True, stop=True,
                        )
                        nc.scalar.activation(
                            out=xt[:], in_=pz[:],
                            func=mybir.ActivationFunctionType.Relu,
                            bias=s32("b1p", hh, hh + 1), scale=1.0,
                        )
                    pp = ppp.tile([P, OUT], f32, tag="pp")
                    nc.tensor.matmul(pp[:], lhsT=x1a[:], rhs=s16("w2p", 0, OUT),
                                     start=True, stop=False)
                    nc.tensor.matmul(pp[:], lhsT=x1b[:],
                                     rhs=s16("w2p", OUT, 2 * OUT),
                                     start=False, stop=True)
                    p2s = wp.tile([P, 2 * OUT], bf16, tag="p2s")
                    nc.vector.tensor_scalar(
                        out=p2s[:, :OUT], in0=pp[:],
                        scalar1=s32("noutcol", b, b + 1), scalar2=None,
                        op0=mybir.AluOpType.mult,
                    )
                    nc.vector.memset(p2s[:, OUT:], 0.0)
                    nc.sync.dma_start(
                        out=p2_shard[b * P : b * P + rows, :],
                        in_=p2s[:rows, :])

                if "ag" in stages:
                    for t in range(NBUCK):
                        nc.gpsimd.collective_compute(
                            "AllGather",
                            mybir.AluOpType.bypass,
                            replica_groups=[list(range(NCORES))],
                            ins=[p2_shard[t * SUB : (t + 1) * SUB, :].opt()],
                            outs=[p2_full[t * NCORES * SUB :
                                          (t + 1) * NCORES * SUB, :].opt()],
                        )

                # ---------- layer 2: dma_gather from p2_full --------------
                elem = 2 * OUT
                slot0 = 0
                ch0 = 0
                l2_mode = ("l2" if "l2" in stages else
                           "l2m" if "l2m" in stages else
                           "l2g" if "l2g" in stages else None)
                for g in range(ngrp if l2_mode else 0):
                    nb = grp_nb[g]
                    b0 = g * GB
                    compute = l2_mode == "l2"
                    masks = l2_mode in ("l2", "l2m")
                    aggs_psum = [
                        pap.tile([P, OUT], f32, tag="pa",
                                 name=f"pa_l2_g{g}_{bl}_r{_rep}")
                        for bl in range(nb)
                    ] if compute else None
                    qsum = int(Q2[g].sum())
                    ch_base = ch0
                    sl_base = slot0
                    for t in range(NBUCK):
                        nch = nb * int(Q2[g, t])
                        ni = nch * P
                        xg = x2p.tile([P, max_slab2 * elem], bf16, tag="xg")
                        tab_ap = p2_full[t * BUCK : (t + 1) * BUCK, :]
                        sub0 = 0
                        for snch in subsplit(nch):
                            sni = snch * P
                            sb0 = sl_base + sub0 * P
                            nc.gpsimd.dma_gather(
                                out_ap=xg[:, sub0 * elem : (sub0 + snch) * elem]
                                    .rearrange("p (c d) -> p c d", d=elem),
                                in_ap=tab_ap,
                                idxs_ap=s16("idx", sb0 // 16, (sb0 + sni) // 16,
                                            cast=False),
                                num_idxs=sni,
                                num_idxs_reg=size_regs[sni],
                                elem_size=elem,
                                single_packet=single_packet,
                                queue_num=qctr[0] % 4,
                            )
                            qctr[0] += 1
                            sub0 += snch
                        if masks:
                            mb = mp.tile([P, iota_w * P], bf16, tag="m")
                            nc.vector.tensor_tensor(
                                out=mb[:, : nch * P]
                                    .rearrange("p (c d) -> p c d", d=P),
                                in0=s16("iotaw", 0, nch * P)
                                    .rearrange("p (c d) -> p c d", d=P),
                                in1=s16("dstloc2", ch_base, ch_base + nch)
                                    .rearrange("p (c one) -> p c one", one=1)
                                    .to_broadcast([P, nch, P]),
                                op=mybir.AluOpType.is_equal,
                            )
                        if compute:
                            for bl in range(nb):
                                for c in range(int(Q2[g, t])):
                                    ci = bl * int(Q2[g, t]) + c
                                    first = t == 0 and c == 0
                                    last = (t == NBUCK - 1
                                            and c == int(Q2[g, t]) - 1)
                                    nc.tensor.matmul(
                                        aggs_psum[bl][:],
                                        lhsT=mb[:, ci * P : (ci + 1) * P],
                                        rhs=xg[:, ci * elem : ci * elem + OUT],
                                        start=first, stop=last,
                                    )
                        ch_base += nch
                        sl_base += ni
                    ch0 += nb * qsum
                    slot0 += nb * qsum * P
                    for bl in range(nb if compute else 0):
                        b = b0 + bl
                        rows = P if b < NBLK - 1 else LAST_ROWS
                        outs = wp.tile([P, OUT], f32, tag="outs")
                        nc.vector.tensor_scalar(
                            out=outs[:], in0=aggs_psum[bl][:],
                            scalar1=s32("nincol", b, b + 1), scalar2=None,
                            op0=mybir.AluOpType.mult,
                        )
                        nc.vector.tensor_tensor(
                            out=outs[:], in0=outs[:], in1=s32("b2bc", 0, OUT),
                            op=mybir.AluOpType.add,
                        )
                        nc.sync.dma_start(
                            out=out_d[b * P : b * P + rows, :],
                            in_=outs[:rows, :])

            for _rep in range(repeat):
                body(_rep)

    nc.finalize()
    return nc


def run_on_device(in_maps, quotas, trace=False):
    from concourse.bass_utils import run_bass_kernel_spmd

    nc = _build_program(quotas)
    return run_bass_kernel_spmd(nc, in_maps, core_ids=list(range(NCORES)),
                                trace=trace)


def kernel(h, src, dst, W1, b1, W2, b2):
    h = np.asarray(h, dtype=np.float32)
    src = np.asarray(src, dtype=np.int32)
    dst = np.asarray(dst, dtype=np.int32)
    W1 = np.asarray(W1, dtype=np.float32)
    b1 = np.asarray(b1, dtype=np.float32)
    W2 = np.asarray(W2, dtype=np.float32)
    b2 = np.asarray(b2, dtype=np.float32)

    quotas, sizes, in_maps = _host_prep(h, src, dst, W1, b1, W2, b2)
    res = run_on_device(in_maps, quotas)
    shards = [r["out"].astype(np.float32) for r in res.results]
    return np.concatenate(shards, axis=0)


# revision 44
# speedup vs baseline: 1.6101x; 1.0540x over previous
"""Distributed 2-layer GCN (DGL GraphConv x2 + ReLU) on 8 Trainium2 NeuronCores.

Strategy (1D dst-node partitioning):
  - Core k owns dst nodes [k*12500, (k+1)*12500). Host buckets edges by dst
    partition and sorts by dst block (128 dst nodes per block).
  - Layer 1: the message rows hpre[src] (hpre = h * out_norm, bf16) depend
    only on the inputs, so the host pre-gathers them into a per-core slot
    stream laid out [128 part, chunk, feat] per dst block. The device
    streams each block's slab contiguously (HWDGE, line rate) and
    segment-sums via one-hot matmuls into PSUM (aggT[f,d]), then
    * in_norm, @W1, +b1, relu, @W2, * out_norm -> p2 shard (bf16, padded
    to 128 cols so layer-2 dma_gather rows are 256B).
  - The p2 AllGather is split into 4 sub-collectives over a permuted
    p2_full row layout (see below) so it overlaps the L1 tail and L2 head.
  - Layer 2: p2 depends on runtime values, so it uses dma_gather
    (SWDGE, ~2.3 ns/row): edges sorted by (dst block, src bucket), gather
    p2_full[src] rows + one-hot matmul segment-sum, * in_norm, + b2 ->
    output shard.
  - Quotas (chunks per block / per (group, bucket)) are max-reduced over
    cores so the SPMD instruction stream is identical on all cores; slack
    slots carry dstloc=999 so their one-hot column is all-zero.
  - All small per-core constant tensors are packed into two blobs (one
    int16/bf16, one f32) because each extra input handle costs ~0.1 ms of
    per-call dispatch overhead on this runtime.
"""

import numpy as np
import ml_dtypes

N, E, IN, HID, OUT = 100000, 1600000, 128, 256, 64
NCORES = 8
NLOC = N // NCORES            # 12500
P = 128
NBLK = (NLOC + P - 1) // P    # 98
LAST_ROWS = NLOC - (NBLK - 1) * P  # 84
BF16 = ml_dtypes.bfloat16
NBUCK = 4
BUCK = 25000                  # L2 bucket size (int16-safe gather indices)
SUB = NLOC // NBUCK           # 3125: p2 sub-shard rows per sub-AllGather
GB = 4                        # dst-blocks per L2 gather group
SUBMAX = 14                   # max chunks per dma_gather (SWDGE ring: <=121
                              # descs per SDMA engine)
# p2_full row layout is PERMUTED so that each of the 4 sub-AllGathers
# produces one contiguous 25000-row bucket: node n (owner k=n//NLOC,
# local l=n%NLOC) lands at row (l//SUB)*8*SUB + k*SUB + (l%SUB). Bucket
# t = rows [t*25000, (t+1)*25000) = sub-AllGather t's output, so layer-2
# gathers for bucket t only wait on sub-collective t (which itself only
# waits on the layer-1 blocks producing shard rows [t*SUB, (t+1)*SUB)).


def _blob_layout(C1, C2, iota_w):
    """Column offsets of the packed constant blobs."""
    o16 = {}
    pos = 0
    TOT2 = C2 * P
    for name, width in (("idx", TOT2 // 16), ("dstloc1", C1),
                        ("dstloc2", C2), ("iotaw", iota_w * P)):
        o16[name] = (pos, width)
        pos += width
    w16 = pos
    o32 = {}
    pos = 0
    for name, width in (("w1", HID), ("w2p", 2 * OUT), ("b1p", 2),
                        ("b2bc", OUT), ("ninT", NBLK * P),
                        ("nincol", NBLK), ("noutcol", NBLK)):
        o32[name] = (pos, width)
        pos += width
    return o16, w16, o32, pos


def _host_prep(h, src, dst, W1, b1, W2, b2):
    deg_in = np.bincount(dst, minlength=N)
    deg_out = np.bincount(src, minlength=N)
    nin = (np.clip(deg_in, 1.0, None) ** -0.5).astype(np.float32)
    nout = (np.clip(deg_out, 1.0, None) ** -0.5).astype(np.float32)

    hpre = (h.astype(np.float32) * nout[:, None]).astype(BF16)

    ngrp = -(-NBLK // GB)
    grp_nb = [min(GB, NBLK - g * GB) for g in range(ngrp)]

    cnt1 = np.zeros((NCORES, NBLK), np.int64)            # L1: per dst block
    cnt2 = np.zeros((NCORES, NBLK, NBUCK), np.int64)     # L2: (block, bucket)
    edges1, edges2 = [], []
    for k in range(NCORES):
        sel = (dst // NLOC) == k
        es = src[sel].astype(np.int64)
        ed = (dst[sel] - k * NLOC).astype(np.int64)
        o1 = np.argsort(ed // P, kind="stable")
        es1, ed1 = es[o1], ed[o1]
        cnt1[k] = np.bincount(ed1 // P, minlength=NBLK)
        edges1.append((es1, ed1))
        # L2: bucket by the permuted p2_full row (see layout comment above)
        sk = es // NLOC
        sl = es % NLOC
        sbu = sl // SUB                      # bucket = sub-AllGather index
        sidx = sk * SUB + sl % SUB           # row within bucket (int16-safe)
        key = (ed // P) * NBUCK + sbu
        o2 = np.argsort(key, kind="stable")
        cnt2[k] = np.bincount(key[o2], minlength=NBLK * NBUCK).reshape(NBLK, NBUCK)
        edges2.append((sidx[o2], ed[o2]))

    # L1 quota: chunks per dst block, max over cores
    q1 = np.maximum(1, -(-cnt1.max(axis=0) // P))        # [NBLK]
    C1 = int(q1.sum())
    c0_1 = np.concatenate([[0], np.cumsum(q1)])

    # L2 quota per (group, bucket): max over cores and blocks-in-group
    Q2 = np.zeros((ngrp, NBUCK), np.int64)
    for g in range(ngrp):
        b0, b1_ = g * GB, min((g + 1) * GB, NBLK)
        Q2[g] = np.maximum(1, -(-cnt2[:, b0:b1_, :].max(axis=(0, 1)) // P))
    C2 = int(sum(grp_nb[g] * Q2[g].sum() for g in range(ngrp)))
    TOT2 = C2 * P
    maxq1 = int(q1.max())
    max_slab2 = GB * int(Q2.max())
    iota_w = max(maxq1, max_slab2)
    o16, w16, o32, w32 = _blob_layout(C1, C2, iota_w)

    iotaw = np.tile(np.tile(np.arange(P, dtype=np.float32),
                            (P, 1)).astype(BF16), (1, iota_w))

    in_maps = []
    for k in range(NCORES):
        # ---- L1: pre-gathered message stream + dstloc
        es1, ed1 = edges1[k]
        bstart = np.concatenate([[0], np.cumsum(cnt1[k])])
        b_of_e = ed1 // P
        pos = np.arange(len(ed1)) - bstart[b_of_e]
        slots1 = (c0_1[b_of_e] * P + pos).astype(np.int64)
        m1 = np.zeros((C1 * P, IN), BF16)
        m1[slots1] = hpre[es1]
        # [C1*P, IN] -> [128 part, C1, IN]: partition-major so each
        # partition's slab read is contiguous
        m1 = np.ascontiguousarray(
            m1.reshape(C1, P, IN).transpose(1, 0, 2).reshape(P, C1 * IN))
        dst1 = np.full(C1 * P, 999.0, np.float32)
        dst1[slots1] = ed1 % P
        dst1 = np.ascontiguousarray(dst1.reshape(C1, P).T).astype(BF16)

        # ---- L2: gather indices + dstloc
        es2, ed2 = edges2[k]
        idx = np.zeros(TOT2, np.int32)
        dst2 = np.full(TOT2, 999.0, np.float32)
        starts = np.concatenate([[0], np.cumsum(cnt2[k].reshape(-1))]).astype(np.int64)
        posn = 0
        for g in range(ngrp):
            for t in range(NBUCK):
                for bl in range(grp_nb[g]):
                    b = g * GB + bl
                    n_bt = int(cnt2[k, b, t])
                    s = int(starts[b * NBUCK + t])
                    idx[posn : posn + n_bt] = es2[s : s + n_bt]
                    dst2[posn : posn + n_bt] = ed2[s : s + n_bt] % P
                    posn += int(Q2[g, t]) * P
        assert posn == TOT2
        # wrap idx int16: slot j of each gather at [j%16, j//16]
        wrap = np.tile(idx.astype(np.int16).reshape(-1, 16).T, (8, 1))
        dst2 = np.ascontiguousarray(dst2.reshape(-1, P).T).astype(BF16)

        nin_loc = nin[k * NLOC : (k + 1) * NLOC]
        nout_loc = nout[k * NLOC : (k + 1) * NLOC]
        pad = NBLK * P - NLOC
        nin_cols = np.ascontiguousarray(
            np.pad(nin_loc, (0, pad)).reshape(NBLK, P).T, dtype=np.float32)
        nout_cols = np.ascontiguousarray(
            np.pad(nout_loc, (0, pad)).reshape(NBLK, P).T, dtype=np.float32)
        nin_tiled = np.tile(np.pad(nin_loc, (0, pad)), (P, 1)).astype(np.float32)

        blob16 = np.empty((P, w16), np.int16)
        for name, arr in (("idx", wrap), ("dstloc1", dst1.view(np.int16)),
                          ("dstloc2", dst2.view(np.int16)),
                          ("iotaw", iotaw.view(np.int16))):
            off, width = o16[name]
            blob16[:, off : off + width] = arr
        blob32 = np.empty((P, w32), np.float32)
        for name, arr in (
                ("w1", np.ascontiguousarray(W1, dtype=np.float32)),
                ("w2p", np.ascontiguousarray(
                    W2.reshape(2, P, OUT).transpose(1, 0, 2)
                    .reshape(P, 2 * OUT), dtype=np.float32)),
                ("b1p", np.ascontiguousarray(b1.reshape(2, P).T,
                                             dtype=np.float32)),
                ("b2bc", np.tile(b2.astype(np.float32), (P, 1))),
                ("ninT", nin_tiled), ("nincol", nin_cols),
                ("noutcol", nout_cols)):
            off, width = o32[name]
            blob32[:, off : off + width] = arr

        in_maps.append({"m1": m1, "blob16": blob16, "blob32": blob32})
    return (q1, Q2), (C1, C2), in_maps


def _build_program(quotas, stages=("l1", "ag", "l2"), repeat=1,
                   single_packet=False, x2bufs=6):
    import concourse.bacc as bacc
    import concourse.mybir as mybir
    import concourse.tile as tile

    q1, Q2 = quotas
    f32 = mybir.dt.float32
    bf16 = mybir.dt.bfloat16
    i16 = mybir.dt.int16

    ngrp = Q2.shape[0]
    grp_nb = [min(GB, NBLK - g * GB) for g in range(ngrp)]
    C1 = int(q1.sum())
    c0_1 = np.concatenate([[0], np.cumsum(q1)])
    C2 = int(sum(grp_nb[g] * Q2[g].sum() for g in range(ngrp)))
    TOT2 = C2 * P
    maxq1 = int(q1.max())
    max_slab2 = GB * int(Q2.max())
    iota_w = max(maxq1, max_slab2)
    o16, w16, o32, w32 = _blob_layout(C1, C2, iota_w)

    nc = bacc.Bacc(None, num_swdge_queues=4)
    qctr = [0]

    def subsplit(nch):
        nsub = -(-nch // SUBMAX)
        base = nch // nsub
        rem = nch - base * nsub
        return [base + (1 if i < rem else 0) for i in range(nsub)]

    sizes = set()
    for g in range(ngrp):
        for t in range(NBUCK):
            for s in subsplit(grp_nb[g] * int(Q2[g, t])):
                sizes.add(s * P)
    size_regs = {s: nc.gpsimd.to_reg(s) for s in sorted(sizes)}

    m1_d = nc.dram_tensor("m1", [P, C1 * IN], bf16, kind="ExternalInput")
    b16_d = nc.dram_tensor("blob16", [P, w16], i16, kind="ExternalInput")
    b32_d = nc.dram_tensor("blob32", [P, w32], f32, kind="ExternalInput")
    out_d = nc.dram_tensor("out", [NLOC, OUT], f32, kind="ExternalOutput")

    with tile.TileContext(nc) as tc:
        with (
            tc.tile_pool(name="const", bufs=1) as constp,
            tc.tile_pool(name="dram", bufs=1, space="DRAM") as dramp,
            tc.tile_pool(name="x1", bufs=3) as x1p,
            tc.tile_pool(name="x2", bufs=x2bufs) as x2p,
            tc.tile_pool(name="mblk", bufs=3) as mp,
            tc.tile_pool(name="work", bufs=3) as wp,
            tc.tile_pool(name="pa", bufs=GB, space="PSUM") as pap,
            tc.tile_pool(name="pz", bufs=2, space="PSUM") as pzp,
            tc.tile_pool(name="pp", bufs=2, space="PSUM") as ppp,
        ):
            b16_sb = constp.tile([P, w16], i16, tag="b16")
            nc.sync.dma_start(out=b16_sb[:], in_=b16_d[:])
            b32_sb = constp.tile([P, w32], f32, tag="b32")
            nc.sync.dma_start(out=b32_sb[:], in_=b32_d[:])

            def s16(name, a, b_, cast=True):
                off, width = o16[name]
                assert 0 <= a and b_ <= width
                ap = b16_sb[:, off + a : off + b_]
                return ap.bitcast(bf16) if cast else ap

            def s32(name, a, b_):
                off, width = o32[name]
                assert 0 <= a and b_ <= width
                return b32_sb[:, off + a : off + b_]

            # p2 stored bf16 padded to 128 cols: 256B rows (dma_gather
            # needs elem_size % 256B == 0); pad half is never read.
            p2_shard = dramp.tile([NLOC, 2 * OUT], bf16, tag="p2s")
            p2_full = dramp.tile([N, 2 * OUT], bf16, tag="p2f")

            def body(_rep):
                # ---------- layer 1: streamed pre-gathered messages -------
                for b in range(NBLK if "l1" in stages else 0):
                    qb = int(q1[b])
                    ch0 = int(c0_1[b])
                    rows = P if b < NBLK - 1 else LAST_ROWS
                    xs = x1p.tile([P, maxq1 * IN], bf16, tag="xs")
                    nc.sync.dma_start(
                        out=xs[:, : qb * IN],
                        in_=m1_d[:, ch0 * IN : (ch0 + qb) * IN])
                    mb = mp.tile([P, iota_w * P], bf16, tag="m")
                    nc.vector.tensor_tensor(
                        out=mb[:, : qb * P].rearrange("p (c d) -> p c d", d=P),
                        in0=s16("iotaw", 0, qb * P)
                            .rearrange("p (c d) -> p c d", d=P),
                        in1=s16("dstloc1", ch0, ch0 + qb)
                            .rearrange("p (c one) -> p c one", one=1)
                            .to_broadcast([P, qb, P]),
                        op=mybir.AluOpType.is_equal,
                    )
                    agg_psum = pap.tile([P, P], f32, tag="pa",
                                        name=f"pa_l1_{b}_r{_rep}")
                    for c in range(qb):
                        nc.tensor.matmul(
                            agg_psum[:],
                            lhsT=xs[:, c * IN : (c + 1) * IN],
                            rhs=mb[:, c * P : (c + 1) * P],
                            start=(c == 0), stop=(c == qb - 1),
                        )
                    # epilogue: * nin, @W1 + b1, relu, @W2, * nout
                    aggs = wp.tile([P, P], f32, tag="aggs")
                    nc.vector.tensor_tensor(
                        out=aggs[:], in0=agg_psum[:],
                        in1=s32("ninT", b * P, (b + 1) * P),
                        op=mybir.AluOpType.mult,
                    )
                    x1a = wp.tile([P, P], f32, tag="x1a")
                    x1b = wp.tile([P, P], f32, tag="x1b")
                    for hh, xt in ((0, x1a), (1, x1b)):
                        pz = pzp.tile([P, P], f32, tag="pz")
                        nc.tensor.matmul(
                            pz[:], lhsT=s32("w1", hh * P, (hh + 1) * P),
                            rhs=aggs[:], start=True, stop=True,
                        )
                        nc.scalar.activation(
                            out=xt[:], in_=pz[:],
                            func=mybir.ActivationFunctionType.Relu,
                            bias=s32("b1p", hh, hh + 1), scale=1.0,
                        )
                    pp = ppp.tile([P, OUT], f32, tag="pp")
                    nc.tensor.matmul(pp[:], lhsT=x1a[:], rhs=s32("w2p", 0, OUT),
                                     start=True, stop=False)
                    nc.tensor.matmul(pp[:], lhsT=x1b[:],
                                     rhs=s32("w2p", OUT, 2 * OUT),
                                     start=False, stop=True)
                    p2s = wp.tile([P, 2 * OUT], bf16, tag="p2s")
                    nc.vector.tensor_scalar(
                        out=p2s[:, :OUT], in0=pp[:],
                        scalar1=s32("noutcol", b, b + 1), scalar2=None,
                        op0=mybir.AluOpType.mult,
                    )
                    nc.vector.memset(p2s[:, OUT:], 0.0)
                    nc.sync.dma_start(
                        out=p2_shard[b * P : b * P + rows, :],
                        in_=p2s[:rows, :])

                if "ag" in stages:
                    for t in range(NBUCK):
                        nc.gpsimd.collective_compute(
                            "AllGather",
                            mybir.AluOpType.bypass,
                            replica_groups=[list(range(NCORES))],
                            ins=[p2_shard[t * SUB : (t + 1) * SUB, :].opt()],
                            outs=[p2_full[t * NCORES * SUB :
                                          (t + 1) * NCORES * SUB, :].opt()],
                        )

                # ---------- layer 2: dma_gather from p2_full --------------
                elem = 2 * OUT
                slot0 = 0
                ch0 = 0
                l2_mode = ("l2" if "l2" in stages else
                           "l2m" if "l2m" in stages else
                           "l2g" if "l2g" in stages else None)
                for g in range(ngrp if l2_mode else 0):
                    nb = grp_nb[g]
                    b0 = g * GB
                    compute = l2_mode == "l2"
                    masks = l2_mode in ("l2", "l2m")
                    aggs_psum = [
                        pap.tile([P, OUT], f32, tag="pa",
                                 name=f"pa_l2_g{g}_{bl}_r{_rep}")
                        for bl in range(nb)
                    ] if compute else None
                    qsum = int(Q2[g].sum())
                    ch_base = ch0
                    sl_base = slot0
                    for t in range(NBUCK):
                        nch = nb * int(Q2[g, t])
                        ni = nch * P
                        xg = x2p.tile([P, max_slab2 * elem], bf16, tag="xg")
                        tab_ap = p2_full[t * BUCK : (t + 1) * BUCK, :]
                        sub0 = 0
                        for snch in subsplit(nch):
                            sni = snch * P
                            sb0 = sl_base + sub0 * P
                            nc.gpsimd.dma_gather(
                                out_ap=xg[:, sub0 * elem : (sub0 + snch) * elem]
                                    .rearrange("p (c d) -> p c d", d=elem),
                                in_ap=tab_ap,
                                idxs_ap=s16("idx", sb0 // 16, (sb0 + sni) // 16,
                                            cast=False),
                                num_idxs=sni,
                                num_idxs_reg=size_regs[sni],
                                elem_size=elem,
                                single_packet=single_packet,
                                queue_num=qctr[0] % 4,
                            )
                            qctr[0] += 1
                            sub0 += snch
                        if masks:
                            mb = mp.tile([P, iota_w * P], bf16, tag="m")
                            nc.vector.tensor_tensor(
                                out=mb[:, : nch * P]
                                    .rearrange("p (c d) -> p c d", d=P),
                                in0=s16("iotaw", 0, nch * P)
                                    .rearrange("p (c d) -> p c d", d=P),
                                in1=s16("dstloc2", ch_base, ch_base + nch)
                                    .rearrange("p (c one) -> p c one", one=1)
                                    .to_broadcast([P, nch, P]),
                                op=mybir.AluOpType.is_equal,
                            )
                        if compute:
                            for bl in range(nb):
                                for c in range(int(Q2[g, t])):
                                    ci = bl * int(Q2[g, t]) + c
                                    first = t == 0 and c == 0
                                    last = (t == NBUCK - 1
                                            and c == int(Q2[g, t]) - 1)
                                    nc.tensor.matmul(
                                        aggs_psum[bl][:],
                                        lhsT=mb[:, ci * P : (ci + 1) * P],
                                        rhs=xg[:, ci * elem : ci * elem + OUT],
                                        start=first, stop=last,
                                    )
                        ch_base += nch
                        sl_base += ni
                    ch0 += nb * qsum
                    slot0 += nb * qsum * P
                    for bl in range(nb if compute else 0):
                        b = b0 + bl
                        rows = P if b < NBLK - 1 else LAST_ROWS
                        outs = wp.tile([P, OUT], f32, tag="outs")
                        nc.vector.tensor_scalar(
                            out=outs[:], in0=aggs_psum[bl][:],
                            scalar1=s32("nincol", b, b + 1), scalar2=None,
                            op0=mybir.AluOpType.mult,
                        )
                        nc.vector.tensor_tensor(
                            out=outs[:], in0=outs[:], in1=s32("b2bc", 0, OUT),
                            op=mybir.AluOpType.add,
                        )
                        nc.sync.dma_start(
                            out=out_d[b * P : b * P + rows, :],
                            in_=outs[:rows, :])

            for _rep in range(repeat):
                body(_rep)

    nc.finalize()
    return nc


def run_on_device(in_maps, quotas, trace=False):
    from concourse.bass_utils import run_bass_kernel_spmd

    nc = _build_program(quotas)
    return run_bass_kernel_spmd(nc, in_maps, core_ids=list(range(NCORES)),
                                trace=trace)


def kernel(h, src, dst, W1, b1, W2, b2):
    h = np.asarray(h, dtype=np.float32)
    src = np.asarray(src, dtype=np.int32)
    dst = np.asarray(dst, dtype=np.int32)
    W1 = np.asarray(W1, dtype=np.float32)
    b1 = np.asarray(b1, dtype=np.float32)
    W2 = np.asarray(W2, dtype=np.float32)
    b2 = np.asarray(b2, dtype=np.float32)

    quotas, sizes, in_maps = _host_prep(h, src, dst, W1, b1, W2, b2)
    res = run_on_device(in_maps, quotas)
    shards = [r["out"].astype(np.float32) for r in res.results]
    return np.concatenate(shards, axis=0)
